# revision 31
# baseline (speedup 1.0000x reference)
"""Trainium2 Bass kernel for nn_DistanceLoss (retrieval_knn, 5-way few-shot
temporal-tuple distance logits).

Math (per the reference):
  tuples = C(8,3) = 56 frame triples; embed dim 1024; supports grouped 5/class.
  qe = relu(q_tuples @ W.T + b); se likewise.
  logits[q,c] = -mean_t min_s ||qe[q,t] - se[c,s]||

Key restructuring on device:
  1. The 6144-wide gather-matmul is factorized: P[j,(a,f)] per frame (7x fewer
     FLOPs), then tuple-combine via run-grouped adds.  Frame columns are
     f-major and tuple columns t-major so every combine op streams contiguous
     50-element (query) / 25-element (support) inner runs.
  2. dist^2 = -2*(dot - q2/2 - s2/2): the norm terms are folded into the dots
     matmul as an extra K=2 accumulation chunk, so the class-min is a plain
     reduce_max over each class's PSUM block.
  3. mean over the 56 tuples is a [128,5].T @ [128,50] block-ones matmul.

Sharding: data-parallel over queries (50/core on 8 cores); support set and
weights replicated; host concatenates the per-core [5,50] outputs.
"""
import sys

sys.path.insert(0, '/opt/trn_rl_repo')
import numpy as np
import ml_dtypes
from itertools import combinations
from contextlib import ExitStack

from concourse import bass, bacc, tile, mybir
from concourse.bass_utils import run_bass_kernel_spmd

BF16 = ml_dtypes.bfloat16
F32 = mybir.dt.float32
BF = mybir.dt.bfloat16
RELU = mybir.ActivationFunctionType.Relu
SQRT = mybir.ActivationFunctionType.Sqrt

WAY, TSS = 5, 3
NS, NQ, SEQ, D = 25, 400, 8, 2048
D2 = 1024
NCORES = 8
QPC = NQ // NCORES              # 50 queries per core
SHOT = NS // WAY                # 5
TUP = list(combinations(range(SEQ), TSS))
TN = len(TUP)                   # 56
QT = QPC * TN                   # 2800 query-tuple columns per core
NMT = (QT + 127) // 128         # 22 M-tiles
QTP = NMT * 128                 # 2816 (padded)
STT = NS * TN                   # 1400 support-tuple columns
SPC = STT // WAY                # 280 per class
KC = D // 128                   # 16 contraction chunks for P
JCN = D2 // 128                 # 8 embed-dim chunks
FQ = QPC * SEQ                  # 400 query frame-columns
FS = NS * SEQ                   # 200 support frame-columns
F = FQ + FS                     # 600
WG = TSS * 128                  # 384 W columns per (kc, jc) group
NRUN = None

# Tuple order is OURS to choose (the class-min and tuple-mean are symmetric
# in tuple order).  Sorting tuples by (f1, f0, f2) makes every group of runs
# with equal f1 a single affine access pattern: for fixed f1, f0 spans
# 0..f1-1 and f2 spans f1+1..7, so one op covers f1*(7-f1) tuples.
# Group (f1): nf0 = f1 runs, each of length ln = 7-f1.
# rbase = number of runs before the group; tbase = number of tuples before.
F1G = []
_rb, _tb = 0, 0
for _f1 in range(1, 7):
    _ln = 7 - _f1
    F1G.append((_f1, _ln, _rb, _tb))
    _rb += _f1
    _tb += _f1 * _ln
NRUN = _rb                      # 21
assert _tb == TN


def build_nc():
    nc = bacc.Bacc("TRN2", target_bir_lowering=False, debug=False)
    # qt: [d, f*50+q], st: [d, f*25+s] (s class-sorted), w: [d, jc*384+a*128+jj]
    qt_d = nc.dram_tensor("qt", [D, FQ], BF, kind="ExternalInput")
    st_d = nc.dram_tensor("st", [D, FS], BF, kind="ExternalInput")
    w_d = nc.dram_tensor("w", [D, TSS * D2], BF, kind="ExternalInput")
    b_d = nc.dram_tensor("b", [128, JCN], F32, kind="ExternalInput")
    bm_d = nc.dram_tensor("bm", [128, NMT * QPC], BF, kind="ExternalInput")
    out_d = nc.dram_tensor("out", [WAY, QPC], F32, kind="ExternalOutput")

    with tile.TileContext(nc) as tc, ExitStack() as ctx:
        ep = ctx.enter_context
        wt_pool = ep(tc.tile_pool(name="wt", bufs=34))
        qst_pool = ep(tc.tile_pool(name="qst", bufs=KC))
        drq_pool = ep(tc.tile_pool(name="drq", bufs=6))
        drs_pool = ep(tc.tile_pool(name="drs", bufs=6))
        s01_pool = ep(tc.tile_pool(name="s01", bufs=2))
        tmp_pool = ep(tc.tile_pool(name="tmp", bufs=2))
        qe_pool = ep(tc.tile_pool(name="qe", bufs=JCN))
        se_pool = ep(tc.tile_pool(name="se", bufs=JCN))
        sq_pool = ep(tc.tile_pool(name="sq", bufs=2))
        row_pool = ep(tc.tile_pool(name="row", bufs=1))
        misc_pool = ep(tc.tile_pool(name="misc", bufs=1))
        eps_pool = ep(tc.tile_pool(name="eps", bufs=3))

        # constants / persistent rows.
        # Norm folding: the class-min is computed on DVE as
        #   mred = min_s(s2 - 2*dot)  (scalar_tensor_tensor + reduce-min)
        # and q2 is added as the per-partition bias of the ACT relu, so no
        # extra PE matmul chunk is needed.
        ones = misc_pool.tile([128, 1], BF, tag="ones")
        nc.vector.memset(ones[:], 1.0)
        ones1 = misc_pool.tile([1, 128], F32, tag="ones1")
        nc.vector.memset(ones1[:], 1.0)
        b_sb = misc_pool.tile([128, JCN], F32, tag="bsb")
        nc.sync.dma_start(b_sb[:], b_d.ap())
        bm_sb = misc_pool.tile([128, NMT * QPC], BF, tag="bm")
        nc.sync.dma_start(bm_sb[:], bm_d.ap())
        s2row = row_pool.tile([1, STT], F32, tag="s2row")
        s2nb = row_pool.tile([128, STT], F32, tag="s2nb")
        q2c = row_pool.tile([128, NMT], F32, tag="q2c")
        sq_total = row_pool.tile([128, QTP], BF, tag="sq_total")
        sqs_total = row_pool.tile([128, STT], BF, tag="sqs_total")

        # frames: queries (cols 0:FQ) and supports (cols FQ:F), K on partitions.
        # jc=0's W tiles are interleaved so the first matmuls start early.
        qst = []
        wtg0 = []
        for kc in range(KC):
            wt = wt_pool.tile([128, WG], BF, tag="wt", name=f"wt0_{kc}")
            nc.sync.dma_start(
                wt[:], w_d.ap()[kc * 128:(kc + 1) * 128, 0:WG])
            wtg0.append(wt)
            t = qst_pool.tile([128, F], BF, tag="qst")
            nc.sync.dma_start(t[:, FQ:F], st_d.ap()[kc * 128:(kc + 1) * 128, :])
            nc.sync.dma_start(t[:, 0:FQ], qt_d.ap()[kc * 128:(kc + 1) * 128, :])
            qst.append(t)

        qe_tiles, se_tiles = [], []
        with tc.tile_pool(name="pp", bufs=3, space="PSUM") as pp_pool, \
             tc.tile_pool(name="ps", bufs=3, space="PSUM") as ps_pool:

            def emit_query_mms(jc, wtg):
                psq = []
                for a in range(TSS):
                    pq = pp_pool.tile([128, FQ], F32, tag="pp",
                                      name=f"pq_{jc}_{a}")
                    for kc in range(KC):
                        nc.tensor.matmul(pq[:], wtg[kc][:, a * 128:(a + 1) * 128],
                                         qst[kc][:, 0:FQ],
                                         start=(kc == 0), stop=(kc == KC - 1))
                    psq.append(pq)
                return psq

            def emit_query_tail(jc, psq):
                bcol = b_sb[:, jc:jc + 1]
                dr = []
                for a in range(TSS):
                    d = drq_pool.tile([128, FQ], BF, tag="drq",
                                      name=f"drq_{jc}_{a}")
                    if a == 0:
                        nc.vector.tensor_scalar_add(d[:], psq[0][:], bcol)
                    else:
                        nc.vector.tensor_copy(d[:], psq[a][:])
                    dr.append(d)
                d0, d1, d2 = dr
                s01q = s01_pool.tile([128, NRUN * QPC], BF, tag="s01q")
                for (f1, ln, rb, tb) in F1G:
                    nc.vector.tensor_add(
                        s01q[:, rb * QPC:(rb + f1) * QPC]
                        .rearrange('p (f q) -> p f q', q=QPC),
                        d0[:, 0:f1 * QPC]
                        .rearrange('p (f q) -> p f q', q=QPC),
                        d1[:, f1 * QPC:(f1 + 1) * QPC]
                        .unsqueeze(1).broadcast_to((128, f1, QPC)))
                tmpq = tmp_pool.tile([128, QT], BF, tag="tmpq")
                for (f1, ln, rb, tb) in F1G:
                    o = tmpq[:, tb * QPC:(tb + f1 * ln) * QPC] \
                        .rearrange('p (f t q) -> p f t q', t=ln, q=QPC)
                    i0 = s01q[:, rb * QPC:(rb + f1) * QPC] \
                        .rearrange('p (f q) -> p f q', q=QPC) \
                        .unsqueeze(2).broadcast_to((128, f1, ln, QPC))
                    i1 = d2[:, (f1 + 1) * QPC:SEQ * QPC] \
                        .rearrange('p (t q) -> p t q', q=QPC) \
                        .unsqueeze(1).broadcast_to((128, f1, ln, QPC))
                    nc.vector.tensor_add(o, i0, i1)
                qe = qe_pool.tile([128, QTP], BF, tag="qe", name=f"qe_{jc}")
                nc.vector.memset(qe[:, QT:QTP], 0.0)
                nc.scalar.activation(qe[:, 0:QT], tmpq[:], RELU)
                qe_tiles.append(qe)
                if jc == 0:
                    nc.scalar.square(sq_total[:], qe[:])
                else:
                    sq = sq_pool.tile([128, QTP], BF, tag="sq")
                    nc.scalar.square(sq[:], qe[:])
                    nc.vector.tensor_add(sq_total[:], sq_total[:], sq[:])

            def emit_support_tail(jc, pss):
                bcol = b_sb[:, jc:jc + 1]
                dr = []
                for a in range(TSS):
                    d = drs_pool.tile([128, FS], BF, tag="drs",
                                      name=f"drs_{jc}_{a}")
                    if a == 0:
                        nc.vector.tensor_scalar_add(d[:], pss[0][:], bcol)
                    else:
                        nc.vector.tensor_copy(d[:], pss[a][:])
                    dr.append(d)
                d0, d1, d2 = dr
                s01s = s01_pool.tile([128, NRUN * NS], BF, tag="s01s")
                for (f1, ln, rb, tb) in F1G:
                    nc.gpsimd.tensor_add(
                        s01s[:, rb * NS:(rb + f1) * NS]
                        .rearrange('p (f q) -> p f q', q=NS),
                        d0[:, 0:f1 * NS]
                        .rearrange('p (f q) -> p f q', q=NS),
                        d1[:, f1 * NS:(f1 + 1) * NS]
                        .unsqueeze(1).broadcast_to((128, f1, NS)))
                tmps = tmp_pool.tile([128, STT], BF, tag="tmps")
                for (f1, ln, rb, tb) in F1G:
                    o = tmps[:, tb * NS:(tb + f1 * ln) * NS] \
                        .rearrange('p (f t q) -> p f t q', t=ln, q=NS)
                    i0 = s01s[:, rb * NS:(rb + f1) * NS] \
                        .rearrange('p (f q) -> p f q', q=NS) \
                        .unsqueeze(2).broadcast_to((128, f1, ln, NS))
                    i1 = d2[:, (f1 + 1) * NS:SEQ * NS] \
                        .rearrange('p (t q) -> p t q', q=NS) \
                        .unsqueeze(1).broadcast_to((128, f1, ln, NS))
                    nc.gpsimd.tensor_add(o, i0, i1)
                sett = tmp_pool.tile([128, STT], BF, tag="sett")
                nc.scalar.activation(sett[:], tmps[:], RELU)
                # reorder t-major -> class-major:
                # out[c*280+sh*56+t] = in[t*25+c*5+sh]
                se = se_pool.tile([128, STT], BF, tag="se", name=f"se_{jc}")
                se_v = se.rearrange('p (c sh t) -> p c sh t', sh=SHOT, t=TN)
                in_v = sett.rearrange('p (t c sh) -> p c sh t', c=WAY, sh=SHOT)
                nc.gpsimd.tensor_copy(se_v, in_v)
                se_tiles.append(se)
                if jc == 0:
                    nc.scalar.square(sqs_total[:], se[:])
                else:
                    sqs = sq_pool.tile([128, STT], BF, tag="sqs")
                    nc.scalar.square(sqs[:], se[:])
                    nc.gpsimd.tensor_add(sqs_total[:], sqs_total[:], sqs[:])

            # Interleave: support MMs for jc, then query MMs for jc-1 (same W
            # tiles, fetched once).  All support-side chains finish well
            # before the dots phase; only qe[7]'s short DVE chain trails.
            wtg_prev, psq_prev = wtg0, None
            for jc in range(JCN):
                if jc == 0:
                    wtg = wtg0
                else:
                    wtg = []
                    for kc in range(KC):
                        wt = wt_pool.tile([128, WG], BF, tag="wt",
                                          name=f"wt{jc}_{kc}")
                        nc.sync.dma_start(
                            wt[:], w_d.ap()[kc * 128:(kc + 1) * 128,
                                            jc * WG:(jc + 1) * WG])
                        wtg.append(wt)
                pss = []
                for a in range(TSS):
                    pg = ps_pool.tile([128, FS], F32, tag="ps",
                                      name=f"pg_{jc}_{a}")
                    for kc in range(KC):
                        nc.tensor.matmul(pg[:], wtg[kc][:, a * 128:(a + 1) * 128],
                                         qst[kc][:, FQ:F],
                                         start=(kc == 0), stop=(kc == KC - 1))
                    pss.append(pg)
                if jc >= 1:
                    psq_prev = emit_query_mms(jc - 1, wtg_prev)
                emit_support_tail(jc, pss)
                if jc >= 1:
                    emit_query_tail(jc - 1, psq_prev)
                wtg_prev = wtg
            psq_last = emit_query_mms(JCN - 1, wtg_prev)
            emit_query_tail(JCN - 1, psq_last)

        # ---- dots + class-min + sqrt + tuple-mean ----
        with tc.tile_pool(name="pd", bufs=6, space="PSUM") as pd_pool, \
             tc.tile_pool(name="pl", bufs=1, space="PSUM") as pl_pool, \
             tc.tile_pool(name="on", bufs=1, space="PSUM") as on_pool, \
             tc.tile_pool(name="dsb", bufs=NMT) as dsb_pool, \
             tc.tile_pool(name="scr", bufs=2) as scr_pool:

            def emit_norm_tail():
                # q2 columns per M-tile: out[p,0] = sum_p' sq_total[p', mt*128+p]
                # (the sq slice is the stationary operand, ones the moving one).
                # s2 stays a row and is broadcast to all partitions by a K=1
                # fp32 matmul.  Emitted after mt=0's dots matmuls: nothing on
                # the PE stream ever waits for it.
                for mt in range(NMT):
                    op = on_pool.tile([128, 512], F32, tag="on", name=f"oq_{mt}")
                    nc.tensor.matmul(op[:, 0:1],
                                     sq_total[:, mt * 128:(mt + 1) * 128],
                                     ones[:], start=True, stop=True)
                    nc.vector.tensor_copy(q2c[:, mt:mt + 1], op[:, 0:1])
                for lo in range(0, STT, 512):
                    hi = min(lo + 512, STT)
                    op = on_pool.tile([128, 512], F32, tag="on", name=f"os_{lo}")
                    nc.tensor.matmul(op[0:1, 0:hi - lo], ones[:],
                                     sqs_total[:, lo:hi], start=True, stop=True)
                    nc.vector.tensor_copy(s2row[:, lo:hi], op[0:1, 0:hi - lo])
                for lo in range(0, STT, 512):
                    hi = min(lo + 512, STT)
                    op = on_pool.tile([128, 512], F32, tag="on", name=f"ob_{lo}")
                    nc.tensor.matmul(op[:, 0:hi - lo], ones1[:],
                                     s2row[:, lo:hi], start=True, stop=True)
                    nc.vector.tensor_copy(s2nb[:, lo:hi], op[:, 0:hi - lo])

            dsb_tiles = []
            for mt in range(NMT):
                pds = [pd_pool.tile([128, SPC], F32, tag="pd",
                                    name=f"pd_{mt}_{c}") for c in range(WAY)]
                for kc2 in range(JCN):
                    lhs = qe_tiles[kc2][:, mt * 128:(mt + 1) * 128]
                    for c in range(WAY):
                        nc.tensor.matmul(pds[c][:], lhs,
                                         se_tiles[kc2][:, c * SPC:(c + 1) * SPC],
                                         start=(kc2 == 0), stop=(kc2 == JCN - 1))
                if mt == 0:
                    emit_norm_tail()
                mred = eps_pool.tile([128, WAY], F32, tag="mred")
                scr = scr_pool.tile([128, WAY * SPC], BF, tag="scr",
                                    name=f"scr_{mt}")
                for c in range(WAY):
                    # scr = s2 - 2*dot
                    nc.vector.scalar_tensor_tensor(
                        out=scr[:, c * SPC:(c + 1) * SPC],
                        in0=pds[c][:], scalar=-2.0,
                        in1=s2nb[:, c * SPC:(c + 1) * SPC],
                        op0=mybir.AluOpType.mult, op1=mybir.AluOpType.add)
                # mred[:, c] = min_s(scr[:, c, :]) in one fused reduce
                nc.vector.tensor_reduce(
                    mred[:], scr.rearrange('p (c s) -> p c s', s=SPC),
                    axis=mybir.AxisListType.X, op=mybir.AluOpType.min)
                r1 = eps_pool.tile([128, WAY], F32, tag="r1")
                nc.scalar.activation(r1[:], mred[:], RELU,
                                     bias=q2c[:, mt:mt + 1])
                dsb = dsb_pool.tile([128, WAY], BF, tag="dsb",
                                    name=f"dsb_{mt}")
                nc.scalar.activation(dsb[:], r1[:], SQRT)
                dsb_tiles.append(dsb)
            # tuple-mean at the end so the PE stream never waits on ACT
            lpsum = pl_pool.tile([WAY, QPC], F32, tag="pl")
            for mt in range(NMT):
                nc.tensor.matmul(lpsum[:], dsb_tiles[mt][:],
                                 bm_sb[:, mt * QPC:(mt + 1) * QPC],
                                 start=(mt == 0), stop=(mt == NMT - 1))
            outsb = misc_pool.tile([WAY, QPC], F32, tag="outsb")
            nc.scalar.mul(outsb[:], lpsum[:], -1.0 / TN)
            nc.sync.dma_start(out_d.ap(), outsb[:])

    nc.compile()
    return nc


_NC = None


def _get_nc():
    global _NC
    if _NC is None:
        _NC = build_nc()
    return _NC


def _host_prep(support_set, support_labels, queries, W, b):
    support_set = np.asarray(support_set)
    support_labels = np.asarray(support_labels)
    queries = np.asarray(queries)
    W = np.asarray(W)
    b = np.asarray(b)

    order = np.argsort(support_labels, kind='stable')
    S = support_set[order]                                        # class-major
    # st[d, f*25+s]
    st = np.ascontiguousarray(
        S.transpose(2, 1, 0).reshape(D, FS)).astype(BF16)
    # W[j, a*2048+d] -> w2[d, jc*384 + a*128 + jj],  j = jc*128+jj
    w2 = np.ascontiguousarray(
        W.reshape(JCN, 128, TSS, D).transpose(3, 0, 2, 1).reshape(D, TSS * D2)
    ).astype(BF16)
    bsb = np.ascontiguousarray(b.reshape(JCN, 128).T).astype(np.float32)
    # qt' = t*50 + q  ->  q = qt' % 50
    bm = np.zeros((128, NMT * QPC), np.float32)
    for g in range(QT):
        mt, p = divmod(g, 128)
        bm[p, mt * QPC + g % QPC] = 1.0
    bmh = bm.astype(BF16)
    in_maps = []
    for c in range(NCORES):
        qs = queries[c * QPC:(c + 1) * QPC]
        # qt[d, f*50+q]
        qtc = np.ascontiguousarray(
            qs.transpose(2, 1, 0).reshape(D, FQ)).astype(BF16)
        in_maps.append({"qt": qtc, "st": st, "w": w2, "b": bsb, "bm": bmh})
    return in_maps


def kernel(support_set, support_labels, queries, W, b):
    in_maps = _host_prep(support_set, support_labels, queries, W, b)
    nc = _get_nc()
    res = run_bass_kernel_spmd(nc, in_maps, core_ids=list(range(NCORES)))
    outs = [np.asarray(res.results[c]["out"]).T for c in range(NCORES)]
    return np.ascontiguousarray(np.concatenate(outs, axis=0)).astype(np.float32)
